# revision 36
# baseline (speedup 1.0000x reference)
"""Causal GQA attention (B=2, H=32, KVH=8, N=2048, D=128) on 8 trn2 cores.

Sharding: 64 (batch, q-head) problems; core c gets q-heads [4c, 4c+4) for both
batches (8 independent attention problems per core).  GQA repeat is
`(r kvh)` ordering, so q-head h uses kv-head h % 8 — each per-core q-head is
paired 1:1 with the kv head it needs; no cross-core communication.

Per-core kernel ("S-transposed" flash-style, no online softmax: rows bounded
so exp computed without max subtraction).  The softmax exp is the throughput
bottleneck (ScalarE ACT ~1 elem/lane/cycle), so exp work is SPLIT between the
Scalar engine (native Exp) and the Vector engine (2 custom DVE ucode ops
implementing a Schraudolph-style exp2 with polynomial mantissa correction,
max rel err ~2e-3).  Work placement:
  - PE: QK matmuls (S'^T blocks [128,512], lhsT=K^T block) + PV matmuls
    (lhsT=P^T 128-col chunk, rhs=[V | 1], denominator rides column 128).
    No mask matmuls: the causal mask moved to GpSimd.
  - ScalarE: exp on diagonal chunks + most dense chunks (scale=ln2; K^T is
    pre-scaled by SCALE*log2(e) on the host).
  - DVE: exp2 on selected dense chunks (EXP2_PACK_ANT packs 2^round(y) into
    fp16 bits via int16 value-convert; EXP2_FIN_ANT multiplies by the
    mantissa-correction poly), one batched reciprocal per group, and the
    finalize scale (one broadcast tensor_tensor per group, fp16 out).
  - GpSimd (otherwise idle): zeroes the masked triangle of each diagonal
    128x128 P^T block (multiply by tri01 in SBUF, post-exp pre-PV).
  - fp16 output DMA; host upcasts to fp32.
"""

import sys

sys.path.insert(0, "/opt/trn_rl_repo")

import numpy as np

import concourse.bass as bass
import concourse.mybir as mybir
from concourse import bacc
import concourse.tile as tile
from concourse.bass import broadcast_tensor_aps
from concourse.bass_utils import run_bass_kernel_spmd

P = 128
NSEQ = 2048
D = 128
NH = 8          # (batch, q-head) problems per core
NG = 4          # query groups per head
GI = 512        # query rows per group
NJB = 16        # 128-wide key blocks per head
SCALE = 1.0 / np.sqrt(128.0)
LN2 = float(np.log(2.0))
C_LOG2E = float(SCALE * np.log2(np.e))  # folded into kT on the host

F16 = mybir.dt.float16
F32 = mybir.dt.float32
I16 = mybir.dt.int16
PRIO_OFF = 250  # make S-production (QK matmuls + exp) beat PV in the scheduler

# --- custom DVE exp2 (Schraudolph pack + poly correction) -------------------
MAGIC = float(1.5 * 2**23)
EXP2_A = 0.23925677   # minimax fit of (2^f - 1) ~= f*(A*f + B), f in [-.5,.5]
EXP2_B = 0.70302311


def dve_tiles_for(h):
    """(g, tile-index) handled by the DVE exp for head h; the rest go to
    ScalarE.  ~4.6k cols/head to DVE balances ACT ~116us / DVE ~117us."""
    return {(1, 2), (2, 3), (3, 3), (3, 5), (3, 7)}


_NC_CACHE = {}


def _f32(x):
    return np.float32(x)


def _refA(in0, in1, s0, s1, imm2):
    a = in0.astype(np.float32)
    w = (a + _f32(s0)).astype(np.float32)
    i = (w - _f32(s0)).astype(np.float32)
    return ((i + _f32(s1)) * _f32(imm2)).astype(np.float32)


def _refB(in0, in1, s0, s1, imm2):
    a = in0.astype(np.float32)
    w = (a + _f32(s0)).astype(np.float32)
    i = (w - _f32(s0)).astype(np.float32)
    f = (a - i).astype(np.float32)
    p = (_f32(1.0) + f * (f * _f32(s1) + _f32(imm2))).astype(np.float32)
    return (p * in1.astype(np.float32)).astype(np.float32)


RECIP_C0 = -0.23549792  # Chebyshev seed pair over x*bitcast(~x) in [-4.5,-4]
RECIP_C1 = 2.0017324


def _refC(in0, in1, s0, s1, imm2):
    x = in1.astype(np.float32)
    nx = (~x.view(np.int32)).view(np.float32)
    y0 = (nx * _f32(s0)).astype(np.float32)
    y1 = (y0 * (_f32(s1) - x * y0)).astype(np.float32)
    return (in0.astype(np.float32) * y1).astype(np.float32)


def register_exp_ops():
    """Idempotently append EXP2_PACK_ANT / EXP2_FIN_ANT to the dve_ops
    registry (name->row map, OPS list, CUSTOM_DVE_SPECS)."""
    import concourse.dve_ops as dve_ops
    from concourse.dve_ops import DveOp
    from concourse.dve_spec import C0, C1, C2, One, Spec, Src0, Src1
    from concourse.dve_spec import lower as dve_lower
    from concourse.dve_uop import DveOpSpec

    from concourse.dve_spec import AluOp, Bin

    if "EXP2_PACK_ANT" in dve_ops._SUB_OPCODE_FOR_NAME:
        pack = next(o for o in dve_ops.OPS if o.name == "EXP2_PACK_ANT")
        fin = next(o for o in dve_ops.OPS if o.name == "EXP2_FIN_ANT")
        mr = next(o for o in dve_ops.OPS if o.name == "MUL_RECIP_ANT")
        return pack, fin, mr

    w = Src0 + C0
    i = w - C0
    specA = Spec(body=(i + C1) * C2, reference=_refA)
    w2 = Src0 + C0
    i2 = w2 - C0
    f = Src0 - i2
    specB = Spec(body=((f * C1 + C2) * f + One) * Src1, reference=_refB)
    # out = Src0 * approx(1/Src1): BITWISE_NOT seed + one Newton step
    ny = Bin(AluOp.BITWISE_NOT, Src1, Src1) * C0
    specC = Spec(body=Src0 * (ny * (C1 - Src1 * ny)), reference=_refC)

    ops = []
    for name, spec, rd1 in (
        ("EXP2_PACK_ANT", specA, False),
        ("EXP2_FIN_ANT", specB, True),
        ("MUL_RECIP_ANT", specC, True),
    ):
        row = dve_ops._CUSTOM_DVE_ROW_BASE + len(dve_ops.OPS)
        dve_ops._SUB_OPCODE_FOR_NAME[name] = row
        sha = DveOpSpec(
            name=name, opcode=row, uops=dve_lower(spec, ver="v3"), rd1_en=rd1
        ).sha("v3")
        op = DveOp(name, spec, subdim=False, uops_sha={"v3": sha})
        dve_ops.OPS.append(op)
        dve_ops.CUSTOM_DVE_SPECS[name] = spec
        ops.append(op)
    return ops[0], ops[1], ops[2]


def build_nc(trace_scopes=False):
    opA, opB, opC = register_exp_ops()
    nc = bacc.Bacc("TRN2", target_bir_lowering=False, debug=False, num_devices=8)

    # per-head packed input: [qT (2048) | kT (2048) | vaug (16*129)] per partition
    W_IN = 2 * NSEQ + NJB * (D + 1)
    inp_d = nc.dram_tensor("inp", [NH, P, W_IN], F16, kind="ExternalInput").ap()
    consts_d = nc.dram_tensor("consts", [P, P], F16, kind="ExternalInput").ap()
    o_d = nc.dram_tensor("o", [NH, NSEQ, D], F16, kind="ExternalOutput").ap()

    def dve_exp(Sf, Ef, Pf, width):
        # 2 custom DVE ops: pack 2^round(y) into fp16 bits (int16 value
        # convert of (round(y)+15)*1024), then multiply by the poly.
        nc.vector._custom_dve(
            opA, out=Ef[:, 0:width].bitcast(I16), in0=Sf[:, 0:width],
            s0=MAGIC, s1=15.0, imm2=1024.0,
        )
        nc.vector._custom_dve(
            opB, out=Pf[:, 0:width], in0=Sf[:, 0:width], in1=Ef[:, 0:width],
            s0=MAGIC, s1=EXP2_A, imm2=EXP2_B,
        )

    with tile.TileContext(nc) as tc:
        with (
            tc.tile_pool(name="cst", bufs=1) as cpool,
            tc.tile_pool(name="inp", bufs=4) as inpool,
            tc.tile_pool(name="pt", bufs=10) as ppool,
            tc.tile_pool(name="e16", bufs=6) as epool,
            tc.tile_pool(name="fin", bufs=10) as finpool,
            tc.tile_pool(name="spsum", bufs=3, space="PSUM") as spool,
            tc.tile_pool(name="opsum", bufs=1, space="PSUM") as opool,
        ):
            cst = cpool.tile([P, P], F16)
            nc.sync.dma_start(cst[:], consts_d)
            # tri01: 1 where p <= c (keep), 0 where p > c (masked future)
            tri01 = cst[:, 0:P]

            for h in range(NH):
                dvec = dve_tiles_for(h)
                hin = inpool.tile([P, W_IN], F16, tag="hin")
                if h == 0:
                    # split so group-0's slices (qT[:512], kT[:512], first 4
                    # va blocks) land first and the pipeline starts early
                    cuts = [0, GI, NSEQ, NSEQ + GI, 2 * NSEQ, 2 * NSEQ + 4 * (D + 1), W_IN]
                    order = [(0, 1), (2, 3), (4, 5), (1, 2), (3, 4), (5, 6)]
                    with tc.high_priority(offset=None):
                        for a, b in order[:3]:
                            nc.sync.dma_start(
                                hin[:, cuts[a] : cuts[b]], inp_d[h, :, cuts[a] : cuts[b]]
                            )
                    for a, b in order[3:]:
                        nc.sync.dma_start(
                            hin[:, cuts[a] : cuts[b]], inp_d[h, :, cuts[a] : cuts[b]]
                        )
                else:
                    nc.sync.dma_start(hin[:], inp_d[h])
                qT = hin[:, 0:NSEQ]
                kT = hin[:, NSEQ : 2 * NSEQ]
                va = hin[:, 2 * NSEQ :].rearrange("p (a b) -> p a b", b=D + 1)

                # last head drains lightest-group-last (g0 has only 2 tiles)
                g_order = range(NG) if h < NH - 1 else reversed(range(NG))
                for g in g_order:
                    # O: two 1-bank PSUM tiles (2 subtiles of 129 each) so
                    # the next group's PVs only wait for their own half's
                    # finalize, not the whole group's
                    Oa = opool.tile([P, 2, 129], F32, tag="Oa")
                    Ob = opool.tile([P, 2, 129], F32, tag="Ob")
                    Ohalf = (Oa, Ob)

                    # Tile schema: each S tile is [128, 2 banks, 512] fp32,
                    # holding "segments" (jb, col off, width, r) — r is the
                    # diag index (mask at the leading 128 cols), or None for
                    # a dense block.  Diagonal-first order so the GpSimd tri01
                    # multiplies hide under the dense tiles that follow.
                    #   T0 (diagA): r0@0 (512) | r1@512 (384), r3@896 (128)
                    #   T1: g0: r2@0 (256); g>0: jb0@0 (512) | r2@512 (256)
                    #   T2..: dense pairs (jb odd, jb even), last single.
                    def segs_for(g):
                        d = 4 * g
                        tiles = [[(d + 0, 0, 512, 0), (d + 1, 512, 384, 1),
                                  (d + 3, 896, 128, 3)]]
                        if g == 0:
                            tiles.append([(d + 2, 0, 256, 2)])
                        else:
                            tiles.append([(0, 0, 512, None),
                                          (d + 2, 512, 256, 2)])
                            jb = 1
                            while jb < d:
                                t = [(jb, 0, 512, None)]
                                if jb + 1 < d:
                                    t.append((jb + 1, 512, 512, None))
                                tiles.append(t)
                                jb += 2
                        return tiles

                    tiles = segs_for(g)

                    # PV issue order (per tile: unmasked first, masked last)
                    # determines the per-O-bank start/stop flags: start zeroes
                    # the whole bank, so exactly the first write per bank
                    # starts and the last write per bank stops.
                    def tile_pvs(t):
                        unm, msk = [], []
                        for jb, off, w, r in t:
                            r0 = 0 if r is None else r
                            for ic in range(r0, 4):
                                e = (jb, ic, off + (ic - r0) * P)
                                if r is not None and ic == r0:
                                    msk.append(e)
                                else:
                                    unm.append(e)
                        return unm + msk

                    first, last = {}, {}
                    for t in tiles:
                        for jb, ic, off in tile_pvs(t):
                            first.setdefault(ic // 2, (jb, ic))
                            last[ic // 2] = (jb, ic)

                    def pv(Pf, off, jb, ic, Ohalf=Ohalf, first=first, last=last):
                        b = ic // 2
                        nc.tensor.matmul(
                            Ohalf[b][:, ic % 2, :],
                            Pf[:, off : off + P],
                            va[:, jb, :],
                            start=(first[b] == (jb, ic)),
                            stop=(last[b] == (jb, ic)),
                        )

                    for ti, t in enumerate(tiles):
                        use_dve = (g, ti) in dvec
                        width = t[-1][1] + t[-1][2]
                        with tc.high_priority(offset=PRIO_OFF):
                            S = spool.tile([P, 2, GI], F32, tag="S")
                            Sf = S[:].rearrange("p a b -> p (a b)")
                            # per PSUM bank: first segment starts (zeroes the
                            # bank), last stops
                            for bank in (0, 1):
                                bsegs = [s for s in t if s[1] // GI == bank]
                                for pos, (jb, off, w, r) in enumerate(bsegs):
                                    q0 = g * GI if r is None else g * GI + r * P
                                    nc.tensor.matmul(
                                        Sf[:, off : off + w],
                                        kT[:, jb * P : (jb + 1) * P],
                                        qT[:, q0 : (g + 1) * GI],
                                        start=(pos == 0),
                                        stop=(pos == len(bsegs) - 1),
                                    )
                            Pt = ppool.tile([P, 2, GI], F16, tag="P")
                            Pf = Pt[:].rearrange("p a b -> p (a b)")
                            if use_dve:
                                E = epool.tile([P, 2, GI], F16, tag="E")
                                Ef = E[:].rearrange("p a b -> p (a b)")
                                dve_exp(Sf, Ef, Pf, width)
                            else:
                                nc.scalar.activation(
                                    Pf[:, 0:width],
                                    Sf[:, 0:width],
                                    mybir.ActivationFunctionType.Exp,
                                    scale=LN2,
                                )
                            # causal mask: zero p > c in each diagonal block
                            for jb, off, w, r in t:
                                if r is not None:
                                    nc.gpsimd.tensor_tensor(
                                        Pf[:, off : off + P],
                                        Pf[:, off : off + P],
                                        tri01,
                                        mybir.AluOpType.mult,
                                    )
                        for jb, ic, off in tile_pvs(t):
                            pv(Pf, off, jb, ic)

                    # finalize on DVE, per O half (reciprocal of the 2 rowsum
                    # columns + one broadcast multiply, fp16 out).  High
                    # priority: these unblock the single-buffered O tiles,
                    # and must beat the NEXT groups' DVE exps in queue order.
                    # (A fused mul-recip custom op was tried; the bir verifier
                    # rejects stride-0 broadcast of PSUM APs in InstISA.)
                    rec = finpool.tile([P, 2, 2, 1], F32, tag="rec")
                    osb = finpool.tile([P, 2, 2, D], F16, tag="osb")
                    with tc.high_priority(offset=PRIO_OFF):
                        for b in range(2):
                            nc.vector.reciprocal(
                                rec[:, b], Ohalf[b][:, :, D : D + 1]
                            )
                            b0, b1 = broadcast_tensor_aps(
                                Ohalf[b][:, :, 0:D], rec[:, b]
                            )
                            nc.vector.tensor_tensor(
                                osb[:, b], b0, b1, mybir.AluOpType.mult
                            )
                    nc.sync.dma_start(
                        o_d[h, g * GI : (g + 1) * GI, :].rearrange(
                            "(a p) d -> p a d", p=P
                        ),
                        osb[:].rearrange("p a b d -> p (a b) d"),
                    )
    nc.compile()
    return nc


def _get_nc():
    if "nc" not in _NC_CACHE:
        _NC_CACHE["nc"] = build_nc()
    return _NC_CACHE["nc"]


def make_consts():
    pp = np.arange(P)[:, None]
    ii = np.arange(P)[None, :]
    return np.where(pp <= ii, np.float16(1.0), np.float16(0.0)).astype(np.float16)


def make_in_maps(q, k, v):
    """Shard full inputs into 8 per-core input maps (host-side layout prep)."""
    consts = make_consts()
    W_IN = 2 * NSEQ + NJB * (D + 1)
    in_maps = []
    for c in range(8):
        inp = np.empty((NH, P, W_IN), dtype=np.float16)
        i = 0
        for b in range(2):
            for qh in range(4 * c, 4 * c + 4):
                kvh = qh % 8
                inp[i, :, 0:NSEQ] = q[b, qh].T
                # kT carries the SCALE*log2(e) factor for the exp2 formulation
                inp[i, :, NSEQ : 2 * NSEQ] = (k[b, kvh].T * C_LOG2E).astype(
                    np.float16
                )
                va = inp[i, :, 2 * NSEQ :].reshape(P, NJB, D + 1)
                # v[b,kvh]: [2048, 128] -> [jb, p, d] -> [p, jb, d]
                va[:, :, :D] = v[b, kvh].reshape(NJB, P, D).transpose(1, 0, 2)
                va[:, :, D] = 1.0
                i += 1
        in_maps.append({"inp": inp, "consts": consts})
    return in_maps


def assemble_output(results):
    out = np.empty((2, 32, NSEQ, D), dtype=np.float32)
    for c in range(8):
        o = results[c]["o"]
        i = 0
        for b in range(2):
            for qh in range(4 * c, 4 * c + 4):
                out[b, qh] = o[i].astype(np.float32)
                i += 1
    return out


def _install_ntff_hook():
    """The agent image's antenv lacks axon_hooks; inject a shim so
    run_bass_kernel_spmd(trace=True) can reach the NTFF profiler in
    libaxon_pjrt.so. Only needed for profiling runs."""
    import types

    if "antenv.axon_hooks" in sys.modules:
        return
    mod = types.ModuleType("antenv.axon_hooks")
    _h = [None]
    mod.set_axon_ntff_profile_hook = lambda h: _h.__setitem__(0, h)
    mod.get_axon_ntff_profile_hook = lambda: _h[0]
    sys.modules["antenv.axon_hooks"] = mod
    import antenv

    antenv.axon_hooks = mod
    if "/root/.axon_site" not in sys.path:
        sys.path.insert(0, "/root/.axon_site")
    from trn_agent_boot.trn_boot import _ntff_profile_via_ctypes

    hook = _ntff_profile_via_ctypes("/opt/axon/libaxon_pjrt.so")
    if hook is not None:
        mod.set_axon_ntff_profile_hook(hook)

    # avoid S3-ish artifact upload in this container
    import concourse.bass_utils as bu

    bu.upload_artifacts = lambda tmpdir: tmpdir


def kernel(q, k, v, _trace=False, _trace_kwargs=None):
    q = np.asarray(q, dtype=np.float32)
    k = np.asarray(k, dtype=np.float32)
    v = np.asarray(v, dtype=np.float32)
    assert q.shape == (2, 32, NSEQ, D), q.shape
    assert k.shape == (2, 8, NSEQ, D), k.shape
    assert v.shape == (2, 8, NSEQ, D), v.shape

    nc = _get_nc()
    in_maps = make_in_maps(q, k, v)
    kwargs = {}
    if _trace:
        _install_ntff_hook()
        kwargs["trace"] = True
        kwargs.update(_trace_kwargs or {})
    res = run_bass_kernel_spmd(nc, in_maps, core_ids=list(range(8)), **kwargs)
    out = assemble_output(res.results)
    if _trace:
        return out, res
    return out
